# revision 20
# baseline (speedup 1.0000x reference)
"""2-layer GCN (gnn_message_passing) on 8 Trainium2 NeuronCores — v2.

Key ideas vs the v1 baseline:
  - Layer-1 AllGather eliminated: every core computes the FULL h1 table
    locally (x is replicated input).  x is pre-scaled by dinv[src] on the
    host, packed in table order, and streamed through SBUF in bf16; 792
    bf16 matmuls produce table rows  row1[n] = dinv_n * (x_n @ W1).
  - Biases folded to the destination side:  sum_s dinv_s dinv_d b  =
    b * (dinv_d * sdeg_d)  with  sdeg_d = sum_{s in in(d)} dinv_s  computed
    on the host, so non-own table rows need NO epilogue at all.
  - Table rows store-interleaved (row = 1024*s + 8*p + jj) so each 8-tile
    DMA store moves 2048B-contiguous chunks (full DMA efficiency).
  - Gather padding cut from 1.385x to ~1.05x: the class greedy targets a
    prefix-ceil histogram (class c of a deg-k dst gets (k+3-c)//4 sources)
    and dst tiles group degree-uniform nodes, so per-(tile,class) slot
    counts are nearly exact.  One tensor_reduce per (tile-group, class).
  - Activation func sets used in phase order (Copy block, Relu block, Exp
    block, one Ln) — no per-tile act-table reloads.
  - Layer 2 keeps one collective AllGather of the h2 table (core-major
    layout = AllGather concatenation order; class = core pair).  The two
    tables have different row layouts, so a second idx stream is uploaded
    into the same SBUF buffer while the collective runs.
"""

import os
import sys
import numpy as np
import ml_dtypes

sys.path.insert(0, "/opt/trn_rl_repo")

SAFE_STORES = os.environ.get("K_SAFE_STORES", "0") == "1"
SAFE_COPY = os.environ.get("K_SAFE_COPY", "0") == "1"
SAFE_REDUCE = os.environ.get("K_SAFE_REDUCE", "0") == "1"
NO_PHASE1 = os.environ.get("K_NO_PHASE1", "0") == "1"
NO_GATHER = os.environ.get("K_NO_GATHER", "0") == "1"

N = 100000
E = 1600000
NFEAT, NHID, NCLASS = 128, 64, 40
NCORES = 8
CPN = 12544            # dst nodes per core (98 tiles of 128)
BLK = CPN + 1          # L2 table block rows per core (+1 zero pad row)
NT = CPN // 128        # 98 dst tiles per core
NCLS = 4
CLS_ROWS = 2 * BLK     # 25090 table rows per class (= 2 core blocks)
TBL = NCORES * BLK     # 100360
NTILE_ALL = 792        # phase-1 col tiles (99 stores x 8 tiles)
TBLP = NTILE_ALL * 128  # 101376 padded L1 table rows
PAD1 = CLS_ROWS - 1    # class-local L1 pad row (always unassigned)
PAD2 = CPN             # class-local L2 pad row (core 2c's zero row)
SLOTS = 96             # gather ring buffer slots per call
GB_BUFS = 3
MAXT = 16              # max tiles per call group
F = 64                 # table row width (f32) = 256B
XCH = 4096             # x-stream chunk cols (32 tiles)
NXCH = (TBLP + XCH - 1) // XCH  # 25
NST1 = NTILE_ALL // 8  # 99 phase-1 stores
NST2 = 13              # h2 stores (12 x 8 tiles + 1 x 2 tiles)
NSTO = 25              # out stores (24 x 4 tiles + 1 x 2 tiles)


def _r1_to_xcol(r):
    """L1 table row -> xT column (store-interleave inverse)."""
    s, o = r // 1024, r % 1024
    return 128 * (8 * s + o % 8) + o // 8


def _pos2_of_dst(d):
    """core-local dst position -> L2 table row within the core block."""
    t, p = d // 128, d % 128
    s2 = t // 8
    return np.where(s2 < 12, 1024 * s2 + 8 * p + t % 8, 12288 + 2 * p + (t - 96))


def _ro_of_dst(d):
    """core-local dst position -> row in the `out` parameter."""
    t, p = d // 128, d % 128
    s3 = t // 4
    return np.where(s3 < 24, 512 * s3 + 4 * p + t % 4, 12288 + 2 * p + (t - 96))


def _host_prep(edge_index):
    rng = np.random.default_rng(12345)
    es = edge_index[0].astype(np.int64)
    ed = edge_index[1].astype(np.int64)
    loops = np.arange(N, dtype=np.int64)
    src = np.concatenate([es, loops])          # add_self_loops
    dst = np.concatenate([ed, loops])
    deg = np.bincount(src, minlength=N)        # >= 1 (self loop)
    dinv64 = 1.0 / np.sqrt(deg.astype(np.float64))
    dinv = dinv64.astype(np.float32)
    sdeg = np.bincount(dst, weights=dinv64[src], minlength=N)  # f64
    indeg = np.bincount(dst, minlength=N)      # k_d incl self

    # ---- greedy class assignment of sources ----
    # Rotated per-degree-block ceil windows keep the targets globally
    # feasible (class edge loads balance) while degree-sorted tiles see a
    # uniform per-(tile,class) target.
    q_, r_ = indeg // 4, indeg % 4
    rho = (indeg // 2) % 4
    tgt = q_[:, None] + ((((np.arange(NCLS))[None, :] - rho[:, None]) % 4)
                         < r_[:, None])                         # [N,4]
    order_e = np.argsort(src, kind="stable")
    d_sorted = dst[order_e]
    sptr = np.searchsorted(src[order_e], np.arange(N + 1))
    cap = 2 * CPN
    cnt = np.zeros((N, NCLS), np.int32)
    cls = np.full(N, -1, np.int8)
    szs = np.zeros(NCLS, np.int64)
    BASE = 32.0
    for s in rng.permutation(N):
        dd = d_sorted[sptr[s]:sptr[s + 1]]
        gap = cnt[dd, :] + 1 - tgt[dd, :]
        sc = (BASE ** np.clip(gap, -2, 3)).sum(0)
        sc = sc + (szs >= cap) * 1e30
        c = int(sc.argmin())
        cls[s] = c
        szs[c] += 1
        cnt[dd, c] += 1
    for _ in range(2):           # refinement passes
        for s in rng.permutation(N):
            c0 = cls[s]
            dd = d_sorted[sptr[s]:sptr[s + 1]]
            cnt[dd, c0] -= 1
            szs[c0] -= 1
            gap = cnt[dd, :] + 1 - tgt[dd, :]
            sc = (BASE ** np.clip(gap, -2, 3)).sum(0)
            sc = sc + (szs >= cap) * 1e30
            c = int(sc.argmin())
            cls[s] = c
            szs[c] += 1
            cnt[dd, c] += 1

    # ---- node -> (core, dst position): per class sort by indeg, deal ----
    # Tail-aligned so high-degree tiles line up across all 8 cores.
    blocks = []                    # per core: node id per dst position
    for c in range(NCLS):
        nodes_c = np.flatnonzero(cls == c)
        nodes_c = nodes_c[np.lexsort((cnt[nodes_c].max(1), indeg[nodes_c]))]
        a = np.full(CPN, -1, np.int64)
        b = np.full(CPN, -1, np.int64)
        na = (len(nodes_c) + 1) // 2
        nb = len(nodes_c) // 2
        a[CPN - na:] = nodes_c[0::2]
        b[CPN - nb:] = nodes_c[1::2]
        blocks.append(a)
        blocks.append(b)

    dcore = np.full(N, -1, np.int64)
    dpos = np.full(N, -1, np.int64)
    for k in range(NCORES):
        blk = blocks[k]
        idxs = np.flatnonzero(blk >= 0)
        dcore[blk[idxs]] = k
        dpos[blk[idxs]] = idxs

    # L1 table rows: class-local slot = membership order
    r1 = np.full(N, -1, np.int64)
    for c in range(NCLS):
        nodes_c = np.flatnonzero(cls == c)
        r1[nodes_c] = c * CLS_ROWS + np.arange(len(nodes_c))
    # L2 table rows (core-major, forced by AllGather)
    r2loc = _pos2_of_dst(dpos)                   # within-core block row

    # ---- per-(core,tile,class,partition) in-neighbor counts ----
    ecore = dcore[dst]
    epos = dpos[dst]
    etile = epos // 128
    epart = epos % 128
    ecls = cls[src].astype(np.int64)
    esrow1 = r1[src] - ecls * CLS_ROWS
    esrow2 = (dcore[src] % 2) * BLK + r2loc[src]
    assert esrow1.min() >= 0 and esrow1.max() < CLS_ROWS
    assert esrow2.min() >= 0 and esrow2.max() < CLS_ROWS

    key = ((ecore * NT + etile) * NCLS + ecls) * 128 + epart
    eorder = np.argsort(key, kind="stable")
    key_s = key[eorder]
    esrow1_s = esrow1[eorder]
    esrow2_s = esrow2[eorder]
    epart_s = epart[eorder]
    counts = np.bincount(key_s, minlength=NCORES * NT * NCLS * 128)
    counts = counts.reshape(NCORES, NT, NCLS, 128)
    kmax = counts.max(axis=(0, 3))                # [NT, NCLS] shared K
    kmax = np.maximum(kmax, 1)

    # ---- per-class tile grouping: uniform K per call, padding-capped ----
    PADCAP = 2
    calls = []       # (c, t0, T, K, off, nsl) in issue order (class-major)
    off = 0
    for c in range(NCLS):
        t0 = 0
        while t0 < NT:
            T = 1
            K = int(kmax[t0, c])
            while t0 + T < NT and T < MAXT:
                Kn = max(K, int(kmax[t0 + T, c]))
                if (T + 1) * Kn > SLOTS:
                    break
                if (T + 1) * Kn - (T * K + int(kmax[t0 + T, c])) > PADCAP:
                    break
                K = Kn
                T += 1
            nsl = T * K
            calls.append((c, t0, T, K, off, nsl))
            off += nsl
            t0 += T
    total_slots = off

    # ---- per-core int16 index streams (both layers, canonical order) ----
    flat_counts = counts.reshape(-1)
    starts = np.concatenate([[0], np.cumsum(flat_counts)])[:-1]
    starts = starts.reshape(NCORES, NT, NCLS, 128)

    streams = []
    for k in range(NCORES):
        stream = np.empty((2, total_slots * 128), np.int16)
        stream[0] = PAD1
        stream[1] = PAD2
        for (c, t0, T, K, coff, nsl) in calls:
            for tl in range(T):
                t = t0 + tl
                cnts = counts[k, t, c]
                assert cnts.max() <= K
                st = starts[k, t, c]
                tot = int(cnts.sum())
                if tot:
                    sl = slice(st[0], st[0] + tot)
                    parts = epart_s[sl]
                    jj = np.arange(tot) - np.repeat(st - st[0], cnts)
                    pos = (coff + (tl * K + jj)) * 128 + parts
                    stream[0, pos] = esrow1_s[sl].astype(np.int16)
                    stream[1, pos] = esrow2_s[sl].astype(np.int16)
        # wrap for dma_gather: idxs[p, j] = stream[j*16 + p%16]
        w1 = np.tile(stream[0].reshape(-1, 16).T, (8, 1))   # [128, cols]
        w2 = np.tile(stream[1].reshape(-1, 16).T, (8, 1))
        streams.append(np.concatenate([w1, w2], axis=1))    # [128, 2*cols]

    meta = dict(blocks=blocks, dpos=dpos, dcore=dcore, dinv=dinv, sdeg=sdeg,
                r1=r1, calls=calls, total_slots=total_slots,
                kmax=kmax, cls=cls)
    return meta, streams


def _build_program(meta):
    import concourse.bacc as bacc
    import concourse.bass as bass
    from concourse import mybir
    from concourse.library_config import mlp
    from contextlib import ExitStack

    AF = mybir.ActivationFunctionType
    OP = mybir.AluOpType
    nc = bacc.Bacc("TRN2", target_bir_lowering=False, debug=False)

    calls = meta["calls"]          # (c, t0, T, K, off, nsl) class-major
    total_slots = meta["total_slots"]
    COLS = total_slots * 8                        # int16 idx cols/partition

    # class c L1 gathers need table rows < (c+1)*CLS_ROWS stored (1024/store)
    st_need = [min(NST1, -(-((c + 1) * CLS_ROWS) // 1024)) for c in range(NCLS)]

    xT = nc.declare_dram_parameter("xT", [128, TBLP], mybir.dt.bfloat16, isOutput=False)
    idxp = nc.declare_dram_parameter("idx", [128, 2 * COLS], mybir.dt.int16, isOutput=False)
    w1t = nc.declare_dram_parameter("w1t", [128, NHID], mybir.dt.bfloat16, isOutput=False)
    w2p = nc.declare_dram_parameter("w2p", [NHID, F], mybir.dt.bfloat16, isOutput=False)
    b1b = nc.declare_dram_parameter("b1b", [128, NHID], mybir.dt.float32, isOutput=False)
    b2b = nc.declare_dram_parameter("b2b", [128, F], mybir.dt.float32, isOutput=False)
    dvc2 = nc.declare_dram_parameter("dvc2", [128, NT], mybir.dt.float32, isOutput=False)
    bs1 = nc.declare_dram_parameter("bs1", [128, NT], mybir.dt.float32, isOutput=False)
    dvc = nc.declare_dram_parameter("dvc", [128, NT], mybir.dt.float32, isOutput=False)
    bs2 = nc.declare_dram_parameter("bs2", [128, NT], mybir.dt.float32, isOutput=False)
    idn = nc.declare_dram_parameter("idn", [128, 128], mybir.dt.float32, isOutput=False)
    zro = nc.declare_dram_parameter("zro", [1, F], mybir.dt.float32, isOutput=False)
    outp = nc.declare_dram_parameter("out", [CPN, NCLASS], mybir.dt.float32, isOutput=True)

    t1 = nc.dram_tensor("t1", [TBLP, F], mybir.dt.float32)
    h2_own = nc.dram_tensor("h2_own", [BLK, F], mybir.dt.float32)
    h2_full = nc.dram_tensor("h2_full", [TBL, F], mybir.dt.float32, addr_space="Shared")

    NAUX = 10
    NB1 = NTILE_ALL // 4                          # 198 psum copy batches
    ntile_ch = [min(32, NTILE_ALL - 32 * i) for i in range(NXCH)]
    mm_done = np.cumsum(ntile_ch)                 # tiles done per chunk

    with ExitStack() as stack:
        ec = stack.enter_context
        block = ec(nc.Block())
        x2sb = ec(nc.sbuf_tensor("x2sb", [128, 2, XCH], mybir.dt.bfloat16))
        idx_sb = ec(nc.sbuf_tensor("idx_sb", [128, COLS], mybir.dt.int16))
        w1t_sb = ec(nc.sbuf_tensor("w1t_sb", [128, NHID], mybir.dt.bfloat16))
        w2p_sb = ec(nc.sbuf_tensor("w2p_sb", [NHID, F], mybir.dt.bfloat16))
        b1b_sb = ec(nc.sbuf_tensor("b1b_sb", [128, NHID], mybir.dt.float32))
        b2b_sb = ec(nc.sbuf_tensor("b2b_sb", [128, F], mybir.dt.float32))
        dvc2_sb = ec(nc.sbuf_tensor("dvc2_sb", [128, NT], mybir.dt.float32))
        bs1_sb = ec(nc.sbuf_tensor("bs1_sb", [128, NT], mybir.dt.float32))
        dvc_sb = ec(nc.sbuf_tensor("dvc_sb", [128, NT], mybir.dt.float32))
        bs2_sb = ec(nc.sbuf_tensor("bs2_sb", [128, NT], mybir.dt.float32))
        idn_sb = ec(nc.sbuf_tensor("idn_sb", [128, 128], mybir.dt.float32))
        zro_sb = ec(nc.sbuf_tensor("zro_sb", [1, F], mybir.dt.float32))
        hstage = ec(nc.sbuf_tensor("hstage", [128, 2, 512], mybir.dt.float32))
        gbuf = ec(nc.sbuf_tensor("gbuf", [128, GB_BUFS, SLOTS, F], mybir.dt.float32))
        agg = ec(nc.sbuf_tensor("agg", [128, NT, F], mybir.dt.float32))
        tmpg = ec(nc.sbuf_tensor("tmpg", [128, MAXT * F], mybir.dt.float32))
        zt = ec(nc.sbuf_tensor("zt", [128, 4, F], mybir.dt.float32))
        x2T = ec(nc.sbuf_tensor("x2T", [NHID, 2, 128], mybir.dt.bfloat16))
        h2sb = ec(nc.sbuf_tensor("h2sb", [128, 2, 512], mybir.dt.float32))
        osb = ec(nc.sbuf_tensor("osb", [128, 2, 4 * NCLASS], mybir.dt.float32))
        etmp = ec(nc.sbuf_tensor("etmp", [128, NCLASS], mybir.dt.float32))
        lse = ec(nc.sbuf_tensor("lse", [128, NT], mybir.dt.float32))
        lnsb = ec(nc.sbuf_tensor("lnsb", [128, NT], mybir.dt.float32))
        rmx = ec(nc.sbuf_tensor("rmx", [128, 4], mybir.dt.float32))
        ph1 = ec(nc.psum_tensor("ph1", [128, 4, 256], mybir.dt.float32))
        pT = ec(nc.psum_tensor("pT", [NHID, 2, 128], mybir.dt.float32))
        p2 = ec(nc.psum_tensor("p2", [128, 2, F], mybir.dt.float32))
        semnames = ["s_in", "s_x", "s_idx", "s_mm1", "s_cp1", "s_st1",
                    "s_g", "s_red", "s_z", "s_tp", "s_xc", "s_mm2",
                    "s_h2c", "s_st2", "s_cc", "s_z2", "s_ex", "s_ln",
                    "s_out", "s_sto"]
        sems = {n: ec(nc.semaphore(n)) for n in semnames}
        (s_in, s_x, s_idx, s_mm1, s_cp1, s_st1, s_g, s_red, s_z, s_tp,
         s_xc, s_mm2, s_h2c, s_st2, s_cc, s_z2, s_ex, s_ln, s_out,
         s_sto) = (sems[n] for n in semnames)

        # ---------------- sync engine: aux uploads + stores ----------------
        @block.sync
        def _(se: bass.BassEngine):
            for p_, sb_ in ((w1t, w1t_sb), (w2p, w2p_sb), (b1b, b1b_sb),
                            (b2b, b2b_sb), (dvc2, dvc2_sb), (bs1, bs1_sb),
                            (dvc, dvc_sb), (bs2, bs2_sb), (idn, idn_sb),
                            (zro, zro_sb)):
                se.dma_start(sb_[:], p_[:]).then_inc(s_in, 16)
            if NO_PHASE1:
                for s in range(NST1):
                    se.dma_start(t1[1024 * s:1024 * s + 1, :],
                                 zro_sb[:]).then_inc(s_st1, 16)
            # x chunk uploads interleaved with phase-1 table stores
            def xdma(i):
                ncols = min(XCH, TBLP - XCH * i)
                if i >= 2:
                    se.wait_ge(s_mm1, int(mm_done[i - 2]))
                se.dma_start(x2sb[:, i % 2, :ncols],
                             xT[:, XCH * i:XCH * i + ncols]).then_inc(s_x, 16)
            if not NO_PHASE1:
                xdma(0)
                xdma(1)
                nxt = 2
                for s in range(NST1):
                    while nxt < NXCH and 32 * (nxt - 1) <= 8 * s + 8:
                        xdma(nxt)
                        nxt += 1
                    se.wait_ge(s_cp1, 2 * s + 2)
                    se.dma_start(
                        t1[1024 * s:1024 * (s + 1), :].rearrange(
                            "(p j) f -> p (j f)", p=128),
                        hstage[:, s % 2, :]).then_inc(s_st1, 16)
                while nxt < NXCH:
                    xdma(nxt)
                    nxt += 1
            # zero pad row of the h2 table, then h2 stores
            se.dma_start(h2_own[CPN:CPN + 1, :], zro_sb[:]).then_inc(s_st2, 16)
            for s in range(NST2):
                ntl = min(8, NT - 8 * s)
                se.wait_ge(s_h2c, min(8 * s + ntl, NT))
                se.dma_start(
                    h2_own[1024 * s:1024 * s + 128 * ntl, :].rearrange(
                        "(p j) f -> p (j f)", p=128),
                    h2sb[:, s % 2, :64 * ntl]).then_inc(s_st2, 16)
            # output stores: 4 tiles per DMA
            for s in range(NSTO):
                ntl = min(4, NT - 4 * s)
                se.wait_ge(s_out, min(4 * s + ntl, NT))
                se.dma_start(
                    outp[512 * s:512 * s + 128 * ntl, :].rearrange(
                        "(p j) f -> p (j f)", p=128),
                    osb[:, s % 2, :NCLASS * ntl]).then_inc(s_sto, 16)

        # ------- scalar (Act): x uploads, psum copies, relu, exp, ln -------
        @block.scalar
        def _(sc):
            # phase-3 relu copies pT -> x2T (bf16)
            for t in range(NT):
                sc.wait_ge(s_tp, t + 1)
                sc.activation(x2T[:, t % 2, :], pT[:, t % 2, :],
                              AF.Relu).then_inc(s_xc)
            # L2 softmax: exp with accumulate -> lse column per tile
            for t in range(NT):
                sc.wait_ge(s_z2, t + 1)
                sc.activation(etmp[:, :], agg[:, t, :NCLASS], AF.Exp,
                              accum_out=lse[:, t:t + 1]).then_inc(s_ex)
            sc.wait_ge(s_ex, NT)
            sc.activation(lnsb[:, :], lse[:, :], AF.Ln).then_inc(s_ln)

        # ---------------- tensor engine ----------------
        @block.tensor
        def _(te):
            te.wait_ge(s_in, 16 * NAUX)
            for j in range([] and NTILE_ALL or (0 if NO_PHASE1 else NTILE_ALL)):
                i = j // 32                       # x chunk
                jc = j - 32 * i
                b = j // 4
                if j % 32 == 0:
                    te.wait_ge(s_x, 16 * (i + 1))
                if j % 4 == 0 and b >= 4:
                    te.wait_ge(s_cp1, b - 3)      # ph1 ring (4)
                te.matmul(ph1[:, b % 4, 64 * (j % 4):64 * (j % 4) + 64],
                          x2sb[:, i % 2, 128 * jc:128 * jc + 128],
                          w1t_sb[:]).then_inc(s_mm1)
            # phase 3: transpose + mm2, software pipelined
            for t in range(NT):
                te.wait_ge(s_z, t + 1)
                if t >= 2:
                    te.wait_ge(s_xc, t - 1)       # pT ring (2)
                te.transpose(pT[:, t % 2, :], zt[:, t % 4, :], idn_sb[:]).then_inc(s_tp)
                if t >= 1:
                    te.wait_ge(s_xc, t)
                    if t >= 3:
                        te.wait_ge(s_h2c, t - 2)  # p2 ring (2)
                    te.matmul(p2[:, (t - 1) % 2, :], x2T[:, (t - 1) % 2, :],
                              w2p_sb[:]).then_inc(s_mm2)
            te.wait_ge(s_xc, NT)
            te.wait_ge(s_h2c, NT - 2)
            te.matmul(p2[:, (NT - 1) % 2, :], x2T[:, (NT - 1) % 2, :],
                      w2p_sb[:]).then_inc(s_mm2)

        # ---------- gpsimd: idx uploads, gathers, collective ----------
        @block.gpsimd
        def _(g: bass.BassGpSimd):
            g.load_library(mlp)
            g.dma_start(idx_sb[:], idxp[:, :COLS]).then_inc(s_idx, 16)
            g.wait_ge(s_idx, 16)
            n = 0
            for (c, t0, T, K, off, nsl) in calls:
                g.wait_ge(s_st1, 16 * st_need[c])
                if n >= GB_BUFS:
                    g.wait_ge(s_red, n - GB_BUFS + 1)
                nidx = nsl * 128
                if NO_GATHER:
                    g.dma_start(gbuf[:, n % GB_BUFS, 0, :],
                                t1[0:128, :].rearrange("(p o) f -> p (o f)", p=128)[:, :F]).then_inc(s_g, 16)
                else:
                    g.dma_gather(
                        gbuf[:, n % GB_BUFS, :nsl, :],
                        t1[c * CLS_ROWS:(c + 1) * CLS_ROWS, :],
                        idx_sb[:, off * 8:(off + nsl) * 8],
                        nidx, nidx, F,
                        single_packet=False,
                    ).then_inc(s_g, 16)
                n += 1
            # second idx stream while the collective runs
            g.dma_start(idx_sb[:], idxp[:, COLS:]).then_inc(s_idx, 16)
            g.wait_ge(s_st2, 16 * (NST2 + 1))
            g.collective_compute(
                "AllGather", mybir.AluOpType.bypass,
                replica_groups=[list(range(NCORES))],
                ins=[h2_own[0:BLK, :].opt()],
                outs=[h2_full[:, :].opt()],
            ).then_inc(s_cc)
            g.wait_ge(s_cc, 1)
            g.wait_ge(s_idx, 32)
            for (c, t0, T, K, off, nsl) in calls:
                if n >= GB_BUFS:
                    g.wait_ge(s_red, n - GB_BUFS + 1)
                nidx = nsl * 128
                if NO_GATHER:
                    g.dma_start(gbuf[:, n % GB_BUFS, 0, :],
                                h2_full[0:128, :].rearrange("(p o) f -> p (o f)", p=128)[:, :F]).then_inc(s_g, 16)
                else:
                    g.dma_gather(
                        gbuf[:, n % GB_BUFS, :nsl, :],
                        h2_full[c * CLS_ROWS:(c + 1) * CLS_ROWS, :],
                        idx_sb[:, off * 8:(off + nsl) * 8],
                        nidx, nidx, F,
                        single_packet=False,
                    ).then_inc(s_g, 16)
                n += 1

        # ---------------- vector engine ----------------
        @block.vector
        def _(v: bass.BassVectorEngine):
            v.wait_ge(s_in, 16 * NAUX)
            n = 0

            def h2copy(t):
                v.wait_ge(s_mm2, t + 1)
                if t >= 16:
                    v.wait_ge(s_st2, 16 * (t // 8))  # h2sb ring (2); zro is +1
                v.tensor_copy(h2sb[:, (t // 8) % 2, 64 * (t % 8):64 * (t % 8) + 64],
                              p2[:, t % 2, :]).then_inc(s_h2c)

            hc = 0  # h2 copy cursor (lags z by 2 tiles)
            cb = 0  # phase-1 psum->sbuf copy cursor

            def p1copy(b):
                if NO_PHASE1:
                    return
                ntl = min(4, NTILE_ALL - 4 * b)
                v.wait_ge(s_mm1, min(4 * b + ntl, NTILE_ALL))
                if b >= 4:
                    v.wait_ge(s_st1, 16 * (b // 2 - 1))
                v.tensor_copy(hstage[:, (b // 2) % 2, 256 * (b % 2):256 * (b % 2) + 256],
                              ph1[:, b % 4, :]).then_inc(s_cp1)

            # ---- L1 aggregation (class-major), phase-1 copies interleaved ----
            for (c, t0, T, K, off, nsl) in calls:
                # emit all copies this call's gather may depend on (+ margin)
                need_cb = min(NB1, 2 * st_need[c] + 6)
                while cb < need_cb:
                    p1copy(cb)
                    cb += 1
                v.wait_ge(s_g, 16 * (n + 1))
                if SAFE_REDUCE:
                    for tl in range(T):
                        seg1 = gbuf[:, n % GB_BUFS, tl * K:(tl + 1) * K, :].rearrange("p k f -> p f k")
                        if c == 0:
                            r = v.tensor_reduce(agg[:, t0 + tl, :], seg1,
                                                axis=mybir.AxisListType.X, op=OP.add)
                        else:
                            r = v.tensor_reduce(tmpg[:, :F], seg1,
                                                axis=mybir.AxisListType.X, op=OP.add)
                            v.tensor_add(agg[:, t0 + tl, :], agg[:, t0 + tl, :],
                                         tmpg[:, :F])
                        if tl == T - 1:
                            r.then_inc(s_red)
                else:
                    seg = gbuf[:, n % GB_BUFS, :nsl, :].rearrange(
                        "p (t k) f -> p t f k", t=T, k=K)
                    if c == 0:
                        v.tensor_reduce(agg[:, t0:t0 + T, :], seg,
                                        axis=mybir.AxisListType.X,
                                        op=OP.add).then_inc(s_red)
                    else:
                        v.tensor_reduce(
                            tmpg[:, :T * F].rearrange("p (t f) -> p t f", t=T),
                            seg, axis=mybir.AxisListType.X,
                            op=OP.add).then_inc(s_red)
                        v.tensor_add(
                            agg[:, t0:t0 + T, :].rearrange("p t f -> p (t f)"),
                            agg[:, t0:t0 + T, :].rearrange("p t f -> p (t f)"),
                            tmpg[:, :T * F])
                n += 1
                if c == NCLS - 1:
                    # z = dvc2*agg + bs1*b1  per tile of this call
                    for t in range(t0, t0 + T):
                        if t >= 4:
                            v.wait_ge(s_tp, t - 3)  # zt ring (4)
                        v.tensor_scalar(out=zt[:, t % 4, :], in0=agg[:, t, :],
                                        scalar1=dvc2_sb[:, t:t + 1],
                                        scalar2=None, op0=OP.mult)
                        v.tensor_scalar(out=tmpg[:, :F], in0=b1b_sb[:],
                                        scalar1=bs1_sb[:, t:t + 1],
                                        scalar2=None, op0=OP.mult)
                        v.tensor_add(zt[:, t % 4, :], zt[:, t % 4, :],
                                     tmpg[:, :F]).then_inc(s_z)
                        while hc <= t - 2:
                            h2copy(hc)
                            hc += 1
            while cb < NB1:
                p1copy(cb)
                cb += 1
            # ---- drain remaining h2 psum -> sbuf copies ----
            while hc < NT:
                h2copy(hc)
                hc += 1
            # ---- L2 aggregation (class-major) + softmax head ----
            for (c, t0, T, K, off, nsl) in calls:
                v.wait_ge(s_g, 16 * (n + 1))
                if SAFE_REDUCE:
                    for tl in range(T):
                        seg1 = gbuf[:, n % GB_BUFS, tl * K:(tl + 1) * K, :].rearrange("p k f -> p f k")
                        if c == 0:
                            r = v.tensor_reduce(agg[:, t0 + tl, :], seg1,
                                                axis=mybir.AxisListType.X, op=OP.add)
                        else:
                            r = v.tensor_reduce(tmpg[:, :F], seg1,
                                                axis=mybir.AxisListType.X, op=OP.add)
                            v.tensor_add(agg[:, t0 + tl, :], agg[:, t0 + tl, :],
                                         tmpg[:, :F])
                        if tl == T - 1:
                            r.then_inc(s_red)
                else:
                    seg = gbuf[:, n % GB_BUFS, :nsl, :].rearrange(
                        "p (t k) f -> p t f k", t=T, k=K)
                    if c == 0:
                        v.tensor_reduce(agg[:, t0:t0 + T, :], seg,
                                        axis=mybir.AxisListType.X,
                                        op=OP.add).then_inc(s_red)
                    else:
                        v.tensor_reduce(
                            tmpg[:, :T * F].rearrange("p (t f) -> p t f", t=T),
                            seg, axis=mybir.AxisListType.X,
                            op=OP.add).then_inc(s_red)
                        v.tensor_add(
                            agg[:, t0:t0 + T, :].rearrange("p t f -> p (t f)"),
                            agg[:, t0:t0 + T, :].rearrange("p t f -> p (t f)"),
                            tmpg[:, :T * F])
                n += 1
                if c == NCLS - 1:
                    for t in range(t0, t0 + T):
                        v.tensor_scalar(out=agg[:, t, :NCLASS],
                                        in0=agg[:, t, :NCLASS],
                                        scalar1=dvc_sb[:, t:t + 1],
                                        scalar2=None, op0=OP.mult)
                        v.tensor_scalar(out=tmpg[:, :NCLASS],
                                        in0=b2b_sb[:, :NCLASS],
                                        scalar1=bs2_sb[:, t:t + 1],
                                        scalar2=None, op0=OP.mult)
                        v.tensor_add(agg[:, t, :NCLASS], agg[:, t, :NCLASS],
                                     tmpg[:, :NCLASS])
                        v.tensor_reduce(rmx[:, t % 4:t % 4 + 1],
                                        agg[:, t, :NCLASS],
                                        axis=mybir.AxisListType.X, op=OP.max)
                        v.tensor_scalar(out=agg[:, t, :NCLASS],
                                        in0=agg[:, t, :NCLASS],
                                        scalar1=rmx[:, t % 4:t % 4 + 1],
                                        scalar2=None,
                                        op0=OP.subtract).then_inc(s_z2)
            # ---- final: osb = agg - ln(sum exp) ----
            v.wait_ge(s_ln, 1)
            for t in range(NT):
                if t >= 8:
                    v.wait_ge(s_sto, 16 * (t // 4 - 1))  # osb ring (2)
                v.tensor_scalar(out=osb[:, (t // 4) % 2, NCLASS * (t % 4):NCLASS * (t % 4) + NCLASS],
                                in0=agg[:, t, :NCLASS],
                                scalar1=lnsb[:, t:t + 1], scalar2=None,
                                op0=OP.subtract).then_inc(s_out)

    nc.compile()
    return nc


_LAST_NC = None


def kernel(x, W1, b1, W2, b2, edge_index):
    global _LAST_NC
    from concourse.bass_utils import run_bass_kernel_spmd

    x = np.asarray(x)
    W1 = np.asarray(W1); b1 = np.asarray(b1)
    W2 = np.asarray(W2); b2 = np.asarray(b2)
    edge_index = np.asarray(edge_index)

    meta, streams = _host_prep(edge_index)
    nc = _build_program(meta)
    _LAST_NC = nc

    dinv = meta["dinv"].astype(np.float64)
    sdeg = meta["sdeg"]
    blocks = meta["blocks"]
    r1 = meta["r1"]
    ident = np.eye(128, dtype=np.float32)

    # xT: column _r1_to_xcol(r1[n]) holds dinv[n] * x[n]; rest zero
    xtab = np.zeros((TBLP, NFEAT), np.float32)
    nodes = np.flatnonzero(r1 >= 0)
    xtab[_r1_to_xcol(r1[nodes])] = x[nodes] * dinv[nodes, None].astype(np.float32)
    xT_np = np.ascontiguousarray(xtab.T).astype(ml_dtypes.bfloat16)

    w1t_np = W1.T.astype(ml_dtypes.bfloat16).copy()
    w2p_np = np.zeros((NHID, F), np.float32)
    w2p_np[:, :NCLASS] = W2.T
    w2p_np = w2p_np.astype(ml_dtypes.bfloat16)
    b1b_np = np.tile(b1.astype(np.float32), (128, 1))
    b2b_np = np.zeros((128, F), np.float32)
    b2b_np[:, :NCLASS] = b2

    def col(vals_per_node, k):
        blk = blocks[k]
        out = np.zeros(CPN, np.float64)
        real = blk >= 0
        out[real] = vals_per_node[blk[real]]
        return np.ascontiguousarray(out.reshape(NT, 128).T).astype(np.float32)

    in_maps = []
    for k in range(NCORES):
        in_maps.append({
            "xT": xT_np, "idx": streams[k], "w1t": w1t_np, "w2p": w2p_np,
            "b1b": b1b_np, "b2b": b2b_np,
            "dvc2": col(dinv * dinv, k),
            "bs1": col(dinv * dinv * sdeg, k),
            "dvc": col(dinv, k),
            "bs2": col(dinv * sdeg, k),
            "idn": ident, "zro": np.zeros((1, F), np.float32),
        })

    res = run_bass_kernel_spmd(nc, in_maps, list(range(NCORES)))

    out = np.empty((N, NCLASS), np.float32)
    for k in range(NCORES):
        blk = blocks[k]
        idxs = np.flatnonzero(blk >= 0)
        ro = _ro_of_dst(idxs)
        out[blk[idxs]] = res.results[k]["out"][ro]
    return out
